# revision 2
# baseline (speedup 1.0000x reference)
"""DiscriminativeLoss kernel for 8 trn2 NeuronCores (Bass/Tile).

Sharding: core c handles image b = c//2, pixel half h = c%2 (N_s = 524288
pixels per core).  Device computes, per core:
  pass 1: per-class counts + segment sums over its pixel shard
          (one-hot matmuls on PE, pixels on the contraction axis),
  AllReduce of per-image [10,18] stats across the 8 cores,
  pass 2: per-pixel hinge-distance sums per class, via 3 accumulating
          matmuls building  s - 2*e.C_k - BIG*(lab-k)^2  on PSUM, then
          Relu(+q[k]-BIG*k^2-dvar^2) and Sqrt ACT ops with per-partition
          accumulation.  Wrong-class lanes land exactly at 0 through the
          whole chain, so the class-masked reduction is a plain row sum.
Host: slices inputs, sums the per-core partial hinge sums and does the
final ~500-flop scalar assembly (centers, pair loss, reg loss, totals).
"""

import os
import sys

import numpy as np

sys.path.insert(0, "/opt/trn_rl_repo")
os.environ.setdefault("MYCRO_LOCAL_CACHE", "1")

import ml_dtypes  # noqa: E402

BF16 = ml_dtypes.bfloat16

# problem constants (hardcoded per harness contract)
B, E, H, W = 4, 16, 1024, 1024
NIMG = H * W
NCORES = 8
NPIX = NIMG // 2            # pixels per core
K = 10
DELTA_VAR = 0.5
DELTA_DST = 1.5
A_W, B_W, R_W = 1.0, 1.0, 0.001
BIG = 1024.0
KJ = 80                      # k-major (8k+j) partition layout size

_cache = {}


def _consts(f1):
    """Host-side constant input arrays shared by all cores."""
    # S2: [128, 80] ones block-diag: S2[16j+e, 8k+j] = 1
    s2 = np.zeros((128, KJ), dtype=np.float32)
    for j in range(8):
        for e in range(E):
            for k in range(K):
                s2[16 * j + e, 8 * k + j] = 1.0
    # S3: [16, 80]: row j = lab-row coeff 2*BIG*k ; row 8+j = lab^2 coeff -BIG
    s3 = np.zeros((16, KJ), dtype=np.float32)
    for j in range(8):
        for k in range(K):
            s3[j, 8 * k + j] = 2.0 * BIG * k
            s3[8 + j, 8 * k + j] = -BIG
    # kpat: [128, 10*f1]: kpat[p, k*f1 + f] = k
    kpat = np.zeros((128, K * f1), dtype=np.float32)
    for k in range(K):
        kpat[:, k * f1:(k + 1) * f1] = float(k)
    # jcol: [80, 10]: jcol[8k+j, k] = 1  (collapse j inside k)
    jcol = np.zeros((KJ, K), dtype=np.float32)
    for k in range(K):
        for j in range(8):
            jcol[8 * k + j, k] = 1.0
    # bk: [80, 1] = -BIG*k^2 - dvar^2
    bk = np.zeros((KJ, 1), dtype=np.float32)
    for k in range(K):
        for j in range(8):
            bk[8 * k + j, 0] = -BIG * k * k - DELTA_VAR * DELTA_VAR
    ones128 = np.ones((128, 1), dtype=np.float32)
    id10 = np.eye(K, dtype=np.float32)
    # qsel: [10, 80]: qsel[k, 8k+j] = 1
    qsel = np.zeros((K, KJ), dtype=np.float32)
    for k in range(K):
        for j in range(8):
            qsel[k, 8 * k + j] = 1.0
    return {
        "qsel": qsel,
        "s2": s2.astype(BF16),
        "s3": s3.astype(BF16),
        "kpat": kpat.astype(BF16),
        "jcol": jcol,
        "bk": bk,
        "ones128": ones128,
        "id10": id10,
    }


def build_module(npix=NPIX, f1=512, f2=512):
    """Build the SPMD Bass module (same program on all 8 cores)."""
    import concourse.bass as bass
    import concourse.mybir as mybir
    import concourse.tile as tile
    from concourse import bacc

    f32 = mybir.dt.float32
    bf16 = mybir.dt.bfloat16
    Alu = mybir.AluOpType
    Act = mybir.ActivationFunctionType

    run = npix // 128            # pixel-major run length per partition
    nch1 = run // f1             # pass-1 chunks
    nch2 = npix // (8 * f2)      # pass-2 chunks
    assert run * 128 == npix and nch1 * f1 == run and nch2 * 8 * f2 == npix

    nc = bacc.Bacc(
        "TRN2",
        target_bir_lowering=False,
        debug=False,
        num_devices=NCORES,
    )

    # I/O
    emb_d = nc.dram_tensor("emb", [E, npix], f32, kind="ExternalInput").ap()
    labf_d = nc.dram_tensor("labf", [npix], f32, kind="ExternalInput").ap()
    embbm_d = nc.dram_tensor("embbm", [E * npix // f2, f2], f32,
                             kind="ExternalInput").ap()
    s2_d = nc.dram_tensor("s2", [128, KJ], bf16, kind="ExternalInput").ap()
    s3_d = nc.dram_tensor("s3", [16, KJ], bf16, kind="ExternalInput").ap()
    kpat_d = nc.dram_tensor("kpat", [128, K * f1], bf16, kind="ExternalInput").ap()
    jcol_d = nc.dram_tensor("jcol", [KJ, K], f32, kind="ExternalInput").ap()
    bk_d = nc.dram_tensor("bk", [KJ, 1], f32, kind="ExternalInput").ap()
    bsel_d = nc.dram_tensor("bsel", [K, 4], f32, kind="ExternalInput").ap()
    qsel_d = nc.dram_tensor("qsel", [K, KJ], f32, kind="ExternalInput").ap()
    ones_d = nc.dram_tensor("ones128", [128, 1], f32, kind="ExternalInput").ap()
    id10_d = nc.dram_tensor("id10", [K, K], f32, kind="ExternalInput").ap()

    hpart_d = nc.dram_tensor("hpart", [1, K], f32, kind="ExternalOutput").ap()
    stats_ext = nc.dram_tensor("stats", [K, 4 * 18], f32, kind="ExternalOutput").ap()

    labsq_d = nc.dram_tensor("labsq", [npix], bf16).ap()

    with tile.TileContext(nc) as tc:
        with (
            tc.tile_pool(name="consts", bufs=1) as cp,
            tc.tile_pool(name="p1", bufs=2) as p1,
            tc.tile_pool(name="p2", bufs=3) as p2,
            tc.tile_pool(name="ps2", bufs=2, space="PSUM") as psp,
            tc.tile_pool(name="ps1", bufs=1, space="PSUM") as ps1,
            tc.tile_pool(name="dram", bufs=1, space="DRAM") as dp,
        ):
            # ---- persistent constants ----
            s2_t = cp.tile([128, KJ], bf16)
            nc.sync.dma_start(s2_t[:], s2_d[:])
            s3_t = cp.tile([16, KJ], bf16)
            nc.sync.dma_start(s3_t[:], s3_d[:])
            kpat_t = cp.tile([128, K * f1], bf16)
            nc.sync.dma_start(kpat_t[:], kpat_d[:])
            jcol_t = cp.tile([KJ, K], f32)
            nc.sync.dma_start(jcol_t[:], jcol_d[:])
            bk_t = cp.tile([KJ, 1], f32)
            nc.sync.dma_start(bk_t[:], bk_d[:])
            bsel_t = cp.tile([K, 4], f32)
            nc.sync.dma_start(bsel_t[:], bsel_d[:])
            qsel_t = cp.tile([K, KJ], f32)
            nc.sync.dma_start(qsel_t[:], qsel_d[:])
            ones_t = cp.tile([128, 1], f32)
            nc.sync.dma_start(ones_t[:], ones_d[:])
            id10_t = cp.tile([K, K], f32)
            nc.sync.dma_start(id10_t[:], id10_d[:])

            # ---- labels: pixel-major [128, run], bf16 (values 0..9 exact) ----
            lab_pm = cp.tile([128, run], bf16)
            nc.gpsimd.dma_start(lab_pm[:], labf_d.rearrange("(p c) -> p c", p=128))
            labsq_pm = cp.tile([128, run], bf16)
            nc.vector.tensor_tensor(labsq_pm[:], lab_pm[:], lab_pm[:], op=Alu.mult)
            nc.gpsimd.dma_start(labsq_d.rearrange("(p c) -> p c", p=128), labsq_pm[:])

            # ---- counts: per-class accumulate on pixel-major labels ----
            cnt_acc = cp.tile([128, K], f32)
            cnt_trash = cp.tile([128, run], bf16)
            for k in range(K):
                nc.vector.tensor_scalar(
                    out=cnt_trash[:],
                    in0=lab_pm[:],
                    scalar1=float(k),
                    scalar2=None,
                    op0=Alu.is_equal,
                    op1=Alu.add,
                    accum_out=cnt_acc[:, k:k + 1],
                )
            cnt_ps = ps1.tile([K, 1], f32)
            nc.tensor.matmul(cnt_ps[:], lhsT=cnt_acc[:], rhs=ones_t[:],
                             start=True, stop=True)

            # ---- pass 1: segment sums via per-slab one-hot matmuls ----
            sums_ps = ps1.tile([K, E], f32)
            for c in range(nch1):
                embp = p1.tile([128, E * f1], bf16, tag="embp")
                # dram: emb[e, p*run + c*f1 + f] -> sbuf [p, e*f1 + f]
                src = emb_d.rearrange("e (p c f) -> c p e f", p=128, c=nch1, f=f1)[c]
                nc.gpsimd.dma_start(
                    embp[:].rearrange("p (e f) -> p e f", f=f1), src)
                ohp = p1.tile([128, K * f1], bf16, tag="ohp")
                lab_b = lab_pm[:, c * f1:(c + 1) * f1]
                nc.vector.tensor_tensor(
                    out=ohp[:].rearrange("p (k f) -> p k f", f=f1),
                    in0=lab_b.unsqueeze(1).to_broadcast([128, K, f1]),
                    in1=kpat_t[:].rearrange("p (k f) -> p k f", f=f1),
                    op=Alu.is_equal,
                )
                ohp_v = ohp[:].rearrange("p (k f) -> p f k", f=f1)
                embp_v = embp[:].rearrange("p (e f) -> p f e", f=f1)
                for f in range(f1):
                    nc.tensor.matmul(
                        sums_ps[:],
                        lhsT=ohp_v[:, f, :],
                        rhs=embp_v[:, f, :],
                        start=(c == 0 and f == 0),
                        stop=(c == nch1 - 1 and f == f1 - 1),
                    )

            # ---- stats block [10, 18]: col0 counts, col1..16 sums ----
            stats_blk = cp.tile([K, 18], f32)
            nc.vector.memset(stats_blk[:], 0.0)
            nc.scalar.copy(stats_blk[:, 0:1], cnt_ps[:])
            nc.scalar.copy(stats_blk[:, 1:1 + E], sums_ps[:])

            # scatter to [10, 4*18] weighted by per-core bsel (one-hot on b)
            stats40 = cp.tile([K, 4 * 18], f32)
            for b in range(4):
                nc.vector.tensor_scalar(
                    out=stats40[:, 18 * b:18 * (b + 1)],
                    in0=stats_blk[:],
                    scalar1=bsel_t[:, b:b + 1],
                    scalar2=None,
                    op0=Alu.mult,
                )

            # ---- AllReduce stats across the 8 cores ----
            cc_in = dp.tile([K, 4 * 18], f32)
            cc_out = dp.tile([K, 4 * 18], f32, addr_space="Shared")
            nc.sync.dma_start(cc_in[:], stats40[:])
            nc.gpsimd.collective_compute(
                "AllReduce",
                mybir.AluOpType.add,
                replica_groups=[list(range(NCORES))],
                ins=[cc_in[:].opt()],
                outs=[cc_out[:].opt()],
            )
            stats_all = cp.tile([K, 4 * 18], f32)
            nc.sync.dma_start(stats_all[:], cc_out[:])
            nc.sync.dma_start(stats_ext[:], cc_out[:])

            # ---- own-image stats: myst = sum_b bsel[b]*stats_all[b] ----
            mya = cp.tile([K, 18], f32)
            myb = cp.tile([K, 18], f32)
            nc.vector.tensor_scalar(
                out=mya[:], in0=stats_all[:, 0:18],
                scalar1=bsel_t[:, 0:1], scalar2=None, op0=Alu.mult)
            srcs = [mya, myb]
            for b in range(1, 4):
                si, so = srcs[(b - 1) % 2], srcs[b % 2]
                nc.vector.scalar_tensor_tensor(
                    out=so[:],
                    in0=stats_all[:, 18 * b:18 * (b + 1)],
                    scalar=bsel_t[:, b:b + 1],
                    in1=si[:],
                    op0=Alu.mult,
                    op1=Alu.add,
                )
            myst = srcs[3 % 2]  # = myb

            # ---- centers, q, stationaries for pass 2 ----
            cnt_safe = cp.tile([K, 1], f32)
            nc.vector.tensor_scalar(out=cnt_safe[:], in0=myst[:, 0:1],
                                    scalar1=1.0, scalar2=None, op0=Alu.max)
            rec = cp.tile([K, 1], f32)
            nc.vector.reciprocal(rec[:], cnt_safe[:])
            cmat = cp.tile([K, E], f32)
            nc.vector.tensor_scalar(out=cmat[:], in0=myst[:, 1:1 + E],
                                    scalar1=rec[:, 0:1], scalar2=None,
                                    op0=Alu.mult)
            csq = cp.tile([K, E], f32)
            nc.vector.tensor_tensor(csq[:], cmat[:], cmat[:], op=Alu.mult)
            qv = cp.tile([K, 1], f32)
            nc.vector.tensor_reduce(qv[:], csq[:], mybir.AxisListType.X, Alu.add)

            ct_ps = ps1.tile([E, K], f32)
            nc.tensor.matmul(ct_ps[:], lhsT=cmat[:], rhs=id10_t[:],
                             start=True, stop=True)
            ctb = cp.tile([E, K], bf16)
            nc.scalar.copy(ctb[:], ct_ps[:])

            ctbm = cp.tile([E, K], bf16)
            nc.vector.tensor_scalar(out=ctbm[:], in0=ctb[:], scalar1=-2.0,
                                    scalar2=None, op0=Alu.mult)
            s1_t = cp.tile([128, KJ], bf16)
            nc.vector.memset(s1_t[:], 0.0)
            s1_v = s1_t[:].rearrange("p (k j) -> p j k", j=8)
            for j in range(8):
                nc.gpsimd.dma_start(
                    s1_v[16 * j:16 * (j + 1), j, :], ctbm[:])
            qb_ps = ps1.tile([KJ, 1], f32)
            nc.tensor.matmul(qb_ps[:], lhsT=qsel_t[:], rhs=qv[:],
                             start=True, stop=True)
            qb2 = cp.tile([KJ, 1], f32)
            nc.scalar.activation(qb2[:], qb_ps[:], Act.Identity,
                                 bias=bk_t[:, 0:1], scale=1.0)
            dv2 = cp.tile([KJ, 1], f32)
            nc.vector.memset(dv2[:], DELTA_VAR * DELTA_VAR)

            # ---- pass 2 ----
            uacc = cp.tile([KJ, nch2], f32)
            yacc = cp.tile([KJ, nch2], f32)
            emb2_r = embbm_d.rearrange("(c p) f -> c p f", p=128)
            labf_r = labf_d.rearrange("(c j f) -> c j f", c=nch2, j=8, f=f2)
            labsq_r = labsq_d.rearrange("(c j f) -> c j f", c=nch2, j=8, f=f2)
            for c in range(nch2):
                embB = p2.tile([128, f2], bf16, tag="embB")
                nc.gpsimd.dma_start(embB[:], emb2_r[c])
                e2B = p2.tile([128, f2], bf16, tag="e2B")
                nc.vector.tensor_tensor(e2B[:], embB[:], embB[:], op=Alu.mult)
                lp = p2.tile([16, f2], bf16, tag="lp")
                nc.gpsimd.dma_start(lp[0:8, :], labf_r[c])
                nc.gpsimd.dma_start(lp[8:16, :], labsq_r[c])

                ps2t = psp.tile([KJ, f2], f32, tag="ps2")
                nc.tensor.matmul(ps2t[:], lhsT=s1_t[:], rhs=embB[:],
                                 start=True, stop=False)
                nc.tensor.matmul(ps2t[:], lhsT=s2_t[:], rhs=e2B[:],
                                 start=False, stop=False)
                nc.tensor.matmul(ps2t[:], lhsT=s3_t[:], rhs=lp[:],
                                 start=False, stop=True)

                u_t = p2.tile([KJ, f2], bf16, tag="u")
                nc.scalar.activation(u_t[:], ps2t[:], Act.Relu,
                                     bias=qb2[:, 0:1], scale=1.0,
                                     accum_out=uacc[:, c:c + 1])
                tr_t = p2.tile([KJ, f2], bf16, tag="tr")
                nc.scalar.activation(tr_t[:], u_t[:], Act.Sqrt,
                                     bias=dv2[:, 0:1], scale=1.0,
                                     accum_out=yacc[:, c:c + 1])

            # ---- H assembly: H_p = sum(u) - 2*dvar*sum(y) + 2*dvar^2*Npp ----
            u1 = cp.tile([KJ, 1], f32)
            y1 = cp.tile([KJ, 1], f32)
            nc.vector.tensor_reduce(u1[:], uacc[:], mybir.AxisListType.X, Alu.add)
            nc.vector.tensor_reduce(y1[:], yacc[:], mybir.AxisListType.X, Alu.add)
            hp = cp.tile([KJ, 1], f32)
            nc.vector.scalar_tensor_tensor(
                out=hp[:], in0=y1[:], scalar=-2.0 * DELTA_VAR, in1=u1[:],
                op0=Alu.mult, op1=Alu.add)
            npp = float(f2 * nch2)
            hp2 = cp.tile([KJ, 1], f32)
            nc.vector.tensor_scalar(
                out=hp2[:], in0=hp[:],
                scalar1=2.0 * DELTA_VAR * DELTA_VAR * npp,
                scalar2=None, op0=Alu.add)
            h_ps = ps1.tile([1, K], f32)
            nc.tensor.matmul(h_ps[:], lhsT=hp2[:], rhs=jcol_t[:],
                             start=True, stop=True)
            h_sb = cp.tile([1, K], f32)
            nc.scalar.copy(h_sb[:], h_ps[:])
            nc.sync.dma_start(hpart_d[:], h_sb[:])

    nc.compile()
    return nc


def _block_major(esh, f2):
    """[E, npix] f32 -> [npix//f2, f2] in pass-2 block-major order
    (chunk c, partition 16j+e, col f)."""
    npix = esh.shape[1]
    nch2 = npix // (8 * f2)
    v = esh.reshape(E, nch2, 8, f2).transpose(1, 2, 0, 3)
    return np.ascontiguousarray(v.reshape(E * npix // f2, f2))


def _host_finalize(stats, hsum):
    """stats: [4, 10, 18] float64-ready; hsum: [4, 10] summed hinge partials."""
    lv_l, ld_l, lr_l, valid_l = [], [], [], []
    ids = np.arange(K)
    for b in range(B):
        counts = stats[b, :, 0].astype(np.float64)
        sums = stats[b, :, 1:1 + E].astype(np.float64)
        present = (counts > 0) & (ids > 0)
        presf = present.astype(np.float64)
        safe = np.where(counts > 0, counts, 1.0)
        centers = sums / safe[:, None]
        per_inst = hsum[b].astype(np.float64) / safe
        n_inst = presf.sum()
        lv = float((per_inst * presf).sum() / max(n_inst, 1.0))
        cdiff = centers[:, None, :] - centers[None, :, :]
        csq = (cdiff * cdiff).sum(-1)
        pm = present[:, None] & present[None, :] & (ids[:, None] < ids[None, :])
        cdist = np.sqrt(np.where(pm, csq, 1.0))
        ph = np.square(np.maximum(2.0 * DELTA_DST - cdist, 0.0)) * pm
        n_pairs = pm.sum()
        ld = float(ph.sum() / max(n_pairs, 1.0))
        cn = np.sqrt(np.where(present, (centers * centers).sum(-1), 1.0))
        lr = float((cn * presf).sum() / max(n_inst, 1.0))
        valid = 1.0 if n_inst > 0 else 0.0
        lv_l.append(lv * valid)
        ld_l.append(ld * valid)
        lr_l.append(lr * valid)
        valid_l.append(valid)
    vb = max(sum(valid_l), 1.0)
    loss_var = sum(lv_l) / vb
    loss_dst = sum(ld_l) / vb
    loss_reg = sum(lr_l) / vb
    total = A_W * loss_var + B_W * loss_dst + R_W * loss_reg
    return (
        np.float32(total),
        np.float32(loss_var),
        np.float32(loss_dst),
        np.float32(loss_reg),
    )


def kernel(embedding, ins_label):
    from concourse.bass_utils import run_bass_kernel_spmd

    key = "mod"
    if key not in _cache:
        _cache[key] = build_module()
    nc = _cache[key]

    consts = _consts(512)
    emb_r = np.asarray(embedding, dtype=np.float32).reshape(B, E, NIMG)
    lab_r = np.asarray(ins_label).reshape(B, NIMG).astype(np.float32)

    in_maps = []
    for c in range(NCORES):
        b, h = c // 2, c % 2
        sl = slice(h * NPIX, (h + 1) * NPIX)
        bsel = np.zeros((K, 4), dtype=np.float32)
        bsel[:, b] = 1.0
        m = dict(consts)
        esh = np.ascontiguousarray(emb_r[b, :, sl])
        m["emb"] = esh
        m["embbm"] = _block_major(esh, 512)
        m["labf"] = np.ascontiguousarray(lab_r[b, sl])
        m["bsel"] = bsel
        in_maps.append(m)

    trace = bool(os.environ.get("KERNEL_TRACE"))
    res = run_bass_kernel_spmd(nc, in_maps, core_ids=list(range(NCORES)),
                               trace=trace)
    global LAST_RES
    LAST_RES = res
    stats = (res.results[0]["stats"].astype(np.float64)
             .reshape(K, 4, 18).transpose(1, 0, 2))
    hsum = np.zeros((B, K), dtype=np.float64)
    for c in range(NCORES):
        hsum[c // 2] += res.results[c]["hpart"].astype(np.float64).reshape(K)
    return _host_finalize(stats, hsum)


if __name__ == "__main__":
    # smoke build
    build_module()
    print("build ok")



# revision 25
# speedup vs baseline: 1.5234x; 1.5234x over previous
"""DiscriminativeLoss kernel for 8 trn2 NeuronCores (Bass/Tile), v2.

Sharding: core c handles image b = c//2, pixel half h = c%2 (N = 524288
pixels per core).  Per core:
  pass 1: per-class counts + segment sums over the pixel shard via 4096
          col-tiled fp8 matmuls (128-pixel contraction each, 4 concurrent
          PE column groups), one-hot rhs built on DVE.
  pairwise AllReduce (cores 2b, 2b+1) of the [17, 10] stats block.
  pass 2: per-pixel hinge-distance sums per class with 12 pixel lanes per
          class on 120 PSUM partitions; 3 accumulating matmuls per
          512-column chunk build 64*(s - 2*e.c_k) - 16384*(lab-k)^2 style
          values; DVE ReLU+accum and batched ACT Sqrt+accum produce the
          two per-lane moments; wrong-class lanes contribute exactly 0.
Host: builds fp8/bf16 staged layouts, sums per-core hinge partials, and
does the final ~500-flop scalar assembly.
"""

import os
import sys

import numpy as np

sys.path.insert(0, "/opt/trn_rl_repo")
os.environ.setdefault("MYCRO_LOCAL_CACHE", "1")

import ml_dtypes  # noqa: E402

BF16 = ml_dtypes.bfloat16
FP8 = ml_dtypes.float8_e4m3

# problem constants (hardcoded per harness contract)
B, E, H, W = 4, 16, 1024, 1024
NIMG = H * W
NCORES = 8
NPIX = NIMG // 2             # pixels per core
K = 10
DELTA_VAR = 0.5
DELTA_DST = 1.5
A_W, B_W, R_W = 1.0, 1.0, 0.001

J2 = 12                      # pass-2 pixel lanes per class
KJ2 = K * J2                 # 120 psum partitions, lane q = 12k + j
F2 = 512                     # pass-2 chunk columns
CHPIX = J2 * F2              # pixels per pass-2 chunk
SC = 64.0                    # center scale for fp8 stationary
BETA = 16384.0               # mask scale (= 256 * SC)
EP1 = E + 2                  # pass-1 P1 columns: 16 emb + ones + lab

_cache = {}


def _consts(f1):
    """Host-side constant input arrays shared by all cores."""
    # kpatk: [128, f1*K] fp8: kpatk[p, f*K + k] = k
    kpatk = np.tile(np.arange(K, dtype=np.float32), (128, f1)).astype(FP8)
    # W2: [37, KJ2] bf16 aux stationary: s/lab/lab^2 rows per j, then the
    # bf16-exact -BETA*k^2 mask-bias row (stream row = ones); the small
    # device-computed SC*(|c|^2 - dvar^2) bias rides a 38th device row.
    w2 = np.zeros((3 * J2 + 1, KJ2), dtype=np.float32)
    for j in range(J2):
        for k in range(K):
            q = J2 * k + j
            w2[j, q] = SC
            w2[J2 + j, q] = 2.0 * BETA * k
            w2[2 * J2 + j, q] = -BETA
    for q in range(KJ2):
        w2[3 * J2, q] = -BETA * (q // J2) ** 2
    # gt64: [K, KJ2] f32: 64 at (k, 12k+j)
    gt64 = np.zeros((K, KJ2), dtype=np.float32)
    # jcol: [KJ2, K] f32: 1 at (12k+j, k)
    jcol = np.zeros((KJ2, K), dtype=np.float32)
    for k in range(K):
        for j in range(J2):
            q = J2 * k + j
            gt64[k, q] = SC
            jcol[q, k] = 1.0
    id17 = np.eye(EP1 - 1, dtype=np.float32)
    id10 = np.eye(K, dtype=np.float32)
    id120 = np.eye(KJ2, dtype=np.float32)
    return {
        "kpatk": kpatk,
        "w2": w2.astype(BF16),
        "gt64": gt64,
        "jcol": jcol,
        "id17": id17,
        "id10": id10,
        "id120": id120,
    }


def build_module(npix=NPIX, f1=512, ncores=NCORES, pair_groups=None):
    """Build the SPMD Bass module (same program on all cores)."""
    import concourse.bass as bass  # noqa: F401
    import concourse.mybir as mybir
    import concourse.tile as tile
    from concourse import bacc

    f32 = mybir.dt.float32
    bf16 = mybir.dt.bfloat16
    f8 = mybir.dt.float8e4
    Alu = mybir.AluOpType
    Act = mybir.ActivationFunctionType

    t1 = npix // 128             # pass-1 pixel tiles
    nch1 = t1 // f1              # pass-1 chunks
    assert t1 * 128 == npix and nch1 * f1 == t1
    nch2 = -(-npix // CHPIX)     # pass-2 chunks (padded)
    n2c = nch2 * F2              # pass-2 columns per lane row
    npp = float(n2c)             # pixels per (k, j) lane
    if pair_groups is None:
        pair_groups = [[2 * i, 2 * i + 1] for i in range(ncores // 2)]

    UB = 4                       # pass-2 chunks per ACT sqrt batch
    DB = 4                       # pass-2 chunks per input DMA

    nc = bacc.Bacc(
        "TRN2",
        target_bir_lowering=False,
        debug=False,
        num_devices=ncores,
    )

    # I/O
    p1_d = nc.dram_tensor("p1", [128, t1 * EP1], f8, kind="ExternalInput").ap()
    e2a_d = nc.dram_tensor("e2a", [96, n2c], f8, kind="ExternalInput").ap()
    e2b_d = nc.dram_tensor("e2b", [96, n2c], f8, kind="ExternalInput").ap()
    aux_d = nc.dram_tensor("aux", [3 * J2 + 2, n2c], bf16,
                           kind="ExternalInput").ap()
    kpatk_d = nc.dram_tensor("kpatk", [128, f1 * K], f8, kind="ExternalInput").ap()
    w2_d = nc.dram_tensor("w2", [3 * J2 + 1, KJ2], bf16,
                          kind="ExternalInput").ap()
    gt64_d = nc.dram_tensor("gt64", [K, KJ2], f32, kind="ExternalInput").ap()
    jcol_d = nc.dram_tensor("jcol", [KJ2, K], f32, kind="ExternalInput").ap()
    id17_d = nc.dram_tensor("id17", [EP1 - 1, EP1 - 1], f32,
                            kind="ExternalInput").ap()
    id10_d = nc.dram_tensor("id10", [K, K], f32, kind="ExternalInput").ap()
    id120_d = nc.dram_tensor("id120", [KJ2, KJ2], f32,
                             kind="ExternalInput").ap()

    stats_ext = nc.dram_tensor("stats", [EP1 - 1, K], f32,
                               kind="ExternalOutput").ap()
    hpart_d = nc.dram_tensor("hpart", [1, K], f32, kind="ExternalOutput").ap()

    with tile.TileContext(nc) as tc:
        with (
            tc.tile_pool(name="consts", bufs=1) as cp,
            tc.tile_pool(name="p1", bufs=3) as p1p,
            tc.tile_pool(name="oh", bufs=3) as ohp,
            tc.tile_pool(name="p2", bufs=2) as p2p,
            tc.tile_pool(name="ub", bufs=2) as ubp,
            tc.tile_pool(name="ps2", bufs=2, space="PSUM") as psp,
            tc.tile_pool(name="ps1", bufs=1, space="PSUM") as ps1,
            tc.tile_pool(name="dram", bufs=1, space="DRAM") as dp,
        ):
            # ---- persistent constants ----
            kpatk_t = cp.tile([128, f1 * K], f8)
            nc.sync.dma_start(kpatk_t[:], kpatk_d[:])
            w2_t = cp.tile([3 * J2 + 2, KJ2], bf16)
            nc.sync.dma_start(w2_t[0:3 * J2 + 1, :], w2_d[:])
            gt64_t = cp.tile([K, KJ2], f32)
            nc.sync.dma_start(gt64_t[:], gt64_d[:])
            id120_t = cp.tile([KJ2, KJ2], f32)
            nc.sync.dma_start(id120_t[:], id120_d[:])
            jcol_t = cp.tile([KJ2, K], f32)
            nc.sync.dma_start(jcol_t[:], jcol_d[:])
            id17_t = cp.tile([EP1 - 1, EP1 - 1], f32)
            nc.sync.dma_start(id17_t[:], id17_d[:])
            id10_t = cp.tile([K, K], f32)
            nc.sync.dma_start(id10_t[:], id10_d[:])

            # ---- pass 1: counts+sums via col-tiled one-hot matmuls ----
            psum4 = [ps1.tile([128, K], f32, name=f"psum4_{g}")
                     for g in range(4)]
            for c in range(nch1):
                p1c = p1p.tile([128, f1 * EP1], f8, tag="p1c")
                nc.sync.dma_start(
                    p1c[:], p1_d[:, c * f1 * EP1:(c + 1) * f1 * EP1])
                p1v = p1c[:].rearrange("p (f e) -> p f e", e=EP1)
                oh = ohp.tile([128, f1 * K], f8, tag="oh")
                ohv = oh[:].rearrange("p (f k) -> p f k", k=K)
                nc.vector.tensor_tensor(
                    out=ohv,
                    in0=p1v[:, :, EP1 - 1:EP1].to_broadcast([128, f1, K]),
                    in1=kpatk_t[:].rearrange("p (f k) -> p f k", k=K),
                    op=Alu.is_equal,
                )
                for t in range(f1):
                    g = t % 4
                    nc.tensor.matmul(
                        psum4[g][32 * g:32 * g + EP1 - 1, :],
                        lhsT=p1v[:, t, 0:EP1 - 1],
                        rhs=ohv[:, t, :],
                        start=(c == 0 and t < 4),
                        stop=(c == nch1 - 1 and t >= f1 - 4),
                        tile_position=(0, 32 * g),
                    )

            # ---- stats: 4 col groups -> DRAM -> pairwise AllReduce ----
            cc_in = dp.tile([EP1 - 1, 4 * K], f32)
            cc_out = dp.tile([EP1 - 1, 4 * K], f32)
            sb4 = cp.tile([128, K], f32)
            for g in range(4):
                nc.scalar.copy(sb4[32 * g:32 * g + EP1 - 1, :],
                               psum4[g][32 * g:32 * g + EP1 - 1, :])
                nc.sync.dma_start(
                    cc_in[:, K * g:K * (g + 1)],
                    sb4[32 * g:32 * g + EP1 - 1, :])
            nc.gpsimd.collective_compute(
                "AllReduce",
                mybir.AluOpType.add,
                replica_groups=pair_groups,
                ins=[cc_in[:].opt()],
                outs=[cc_out[:].opt()],
            )
            st40 = cp.tile([EP1 - 1, 4 * K], f32)
            nc.sync.dma_start(st40[:], cc_out[:])
            myst = cp.tile([EP1 - 1, K], f32)
            nc.vector.tensor_reduce(
                myst[:], st40[:].rearrange("p (g k) -> p k g", g=4),
                mybir.AxisListType.X, Alu.add)
            nc.sync.dma_start(stats_ext[:], myst[:])

            # ---- centers & pass-2 stationaries ----
            ps_a = psp.tile([KJ2, F2], f32, tag="ps2", name="ps_a")
            stT_ps = ps_a[0:K, 0:EP1 - 1]
            nc.tensor.transpose(stT_ps, myst[:], id17_t[:])
            stT = cp.tile([K, EP1 - 1], f32)
            nc.scalar.copy(stT[:], stT_ps)
            cnt_safe = cp.tile([K, 1], f32)
            nc.vector.tensor_scalar(
                out=cnt_safe[:], in0=stT[:, E:E + 1], scalar1=1.0,
                scalar2=None, op0=Alu.max)
            rec = cp.tile([K, 1], f32)
            nc.vector.reciprocal(rec[:], cnt_safe[:])
            cmat = cp.tile([K, E], f32)
            nc.vector.tensor_scalar(
                out=cmat[:], in0=stT[:, 0:E], scalar1=rec[:, 0:1],
                scalar2=None, op0=Alu.mult)
            csq = cp.tile([K, E], f32)
            nc.vector.tensor_tensor(csq[:], cmat[:], cmat[:], op=Alu.mult)
            c2 = cp.tile([K, 1], f32)
            nc.vector.tensor_reduce(c2[:], csq[:], mybir.AxisListType.X, Alu.add)
            ps_b = psp.tile([KJ2, F2], f32, tag="ps2", name="ps_b")
            qb_ps = ps_b[:, 0:1]
            nc.tensor.matmul(qb_ps, lhsT=gt64_t[:], rhs=c2[:],
                             start=True, stop=True)
            qsb = cp.tile([KJ2, 1], f32)
            nc.scalar.copy(qsb[:], qb_ps)
            ps_q = psp.tile([KJ2, F2], f32, tag="ps2", name="ps_q")
            qT_ps = ps_q[0:1, 0:KJ2]
            nc.tensor.transpose(qT_ps, qsb[:], id120_t[:])
            qT = cp.tile([1, KJ2], bf16)
            nc.vector.tensor_scalar(
                out=qT[:], in0=qT_ps,
                scalar1=-SC * DELTA_VAR * DELTA_VAR, scalar2=None,
                op0=Alu.add)
            nc.sync.dma_start(w2_t[3 * J2 + 1:3 * J2 + 2, :], qT[:])

            cmm = cp.tile([K, E], f32)
            nc.vector.tensor_scalar(
                out=cmm[:], in0=cmat[:], scalar1=-2.0 * SC, scalar2=None,
                op0=Alu.mult)
            ps_c = psp.tile([KJ2, F2], f32, tag="ps2", name="ps_c")
            cT_ps = ps_c[0:E, 0:K]
            nc.tensor.transpose(cT_ps, cmm[:], id10_t[:])
            cT8 = cp.tile([E, K], f8)
            nc.scalar.copy(cT8[:], cT_ps)
            w1a = cp.tile([96, KJ2], f8)
            w1b = cp.tile([96, KJ2], f8)
            nc.vector.memset(w1a[:], 0.0)
            nc.vector.memset(w1b[:], 0.0)
            w1a_v = w1a[:].rearrange("p (k j) -> p j k", j=J2)
            w1b_v = w1b[:].rearrange("p (k j) -> p j k", j=J2)
            for j in range(6):
                nc.sync.dma_start(w1a_v[16 * j:16 * (j + 1), j, :], cT8[:])
                nc.sync.dma_start(w1b_v[16 * j:16 * (j + 1), 6 + j, :], cT8[:])

            # ---- pass 2 ----
            dv2 = cp.tile([KJ2, 1], f32)
            nc.vector.memset(dv2[:], DELTA_VAR * DELTA_VAR)
            ucol = cp.tile([KJ2, nch2], f32)
            ycol = cp.tile([KJ2, -(-nch2 // UB)], f32)
            ubuf = None
            for c in range(nch2):
                if c % DB == 0:
                    nd = min(DB, nch2 - c) * F2
                    e2a_t = p2p.tile([96, DB * F2], f8, tag="e2a")
                    nc.sync.dma_start(
                        e2a_t[:, 0:nd], e2a_d[:, c * F2:c * F2 + nd])
                    e2b_t = p2p.tile([96, DB * F2], f8, tag="e2b")
                    nc.sync.dma_start(
                        e2b_t[:, 0:nd], e2b_d[:, c * F2:c * F2 + nd])
                    aux_t = p2p.tile([3 * J2 + 2, DB * F2], bf16, tag="aux")
                    nc.sync.dma_start(
                        aux_t[:, 0:nd], aux_d[:, c * F2:c * F2 + nd])
                off = (c % DB) * F2
                ps2 = psp.tile([KJ2, F2], f32, tag="ps2")
                nc.tensor.matmul(ps2[:], lhsT=w1a[:], rhs=e2a_t[:, off:off + F2],
                                 start=True, stop=False)
                nc.tensor.matmul(ps2[:], lhsT=w1b[:], rhs=e2b_t[:, off:off + F2],
                                 start=False, stop=False)
                nc.tensor.matmul(ps2[:], lhsT=w2_t[:], rhs=aux_t[:, off:off + F2],
                                 start=False, stop=True)
                if c % UB == 0:
                    ubuf = ubp.tile([KJ2, UB * F2], bf16, tag="ubuf")
                uo = (c % UB) * F2
                nc.vector.tensor_scalar(
                    out=ubuf[:, uo:uo + F2],
                    in0=ps2[:],
                    scalar1=0.0,
                    scalar2=None,
                    op0=Alu.max,
                    op1=Alu.add,
                    accum_out=ucol[:, c:c + 1],
                )
                if c % UB == UB - 1 or c == nch2 - 1:
                    nu = (c % UB + 1) * F2
                    ytr = ubp.tile([KJ2, UB * F2], bf16, tag="ytr")
                    nc.scalar.activation(
                        ytr[:, 0:nu], ubuf[:, 0:nu], Act.Sqrt,
                        bias=dv2[:, 0:1], scale=1.0 / SC,
                        accum_out=ycol[:, c // UB:c // UB + 1])

            # ---- H assembly ----
            u1 = cp.tile([KJ2, 1], f32)
            y1 = cp.tile([KJ2, 1], f32)
            nc.vector.tensor_reduce(u1[:], ucol[:], mybir.AxisListType.X, Alu.add)
            nc.vector.tensor_reduce(y1[:], ycol[:], mybir.AxisListType.X, Alu.add)
            hp = cp.tile([KJ2, 1], f32)
            nc.vector.scalar_tensor_tensor(
                out=hp[:], in0=y1[:], scalar=-2.0 * DELTA_VAR * SC, in1=u1[:],
                op0=Alu.mult, op1=Alu.add)
            hp2 = cp.tile([KJ2, 1], f32)
            nc.vector.tensor_scalar(
                out=hp2[:], in0=hp[:],
                scalar1=2.0 * DELTA_VAR * DELTA_VAR * SC * npp,
                scalar2=None, op0=Alu.add)
            ps_h = psp.tile([KJ2, F2], f32, tag="ps2", name="ps_h")
            h_ps = ps_h[0:1, 0:K]
            nc.tensor.matmul(h_ps, lhsT=hp2[:], rhs=jcol_t[:],
                             start=True, stop=True)
            h_sb = cp.tile([1, K], f32)
            nc.scalar.copy(h_sb[:], h_ps)
            nc.sync.dma_start(hpart_d[:], h_sb[:])

    nc.compile()
    return nc


def _host_layouts(e_half, lab_half, f1):
    """Build per-core staged arrays: P1 fp8, e2a/e2b fp8, aux bf16."""
    npix = e_half.shape[1]
    t1 = npix // 128
    nch2 = -(-npix // CHPIX)
    n2pad = nch2 * CHPIX - npix

    p1 = np.empty((128, t1, EP1), dtype=FP8)
    p1[:, :, 0:E] = e_half.reshape(E, t1, 128).transpose(2, 1, 0)
    p1[:, :, E] = 1.0
    p1[:, :, E + 1] = lab_half.reshape(t1, 128).T

    e_pad = np.concatenate(
        [e_half, np.zeros((E, n2pad), np.float32)], axis=1)
    y = e_pad.reshape(E, nch2, J2, F2)
    e2a = np.ascontiguousarray(
        y[:, :, 0:6].transpose(2, 0, 1, 3).reshape(96, nch2 * F2)).astype(FP8)
    e2b = np.ascontiguousarray(
        y[:, :, 6:12].transpose(2, 0, 1, 3).reshape(96, nch2 * F2)).astype(FP8)

    s = np.square(e_half).sum(axis=0)
    aux = np.empty((3 * J2 + 2, nch2, F2), dtype=np.float32)
    for i, vec in enumerate((s, lab_half, lab_half * lab_half)):
        v = np.concatenate([vec, np.zeros(n2pad, np.float32)])
        aux[i * J2:(i + 1) * J2] = v.reshape(nch2, J2, F2).transpose(1, 0, 2)
    aux[3 * J2:] = 1.0
    return {
        "p1": p1.reshape(128, t1 * EP1),
        "e2a": e2a,
        "e2b": e2b,
        "aux": aux.reshape(3 * J2 + 2, nch2 * F2).astype(BF16),
    }


def _host_finalize(stats, hsum):
    """stats: [B, 17, 10] (rows 0..15 sums[e,k], row 16 counts);
    hsum: [B, 10] hinge sums (already /SC)."""
    lv_l, ld_l, lr_l, valid_l = [], [], [], []
    ids = np.arange(K)
    for b in range(B):
        counts = stats[b, E, :].astype(np.float64)
        sums = stats[b, 0:E, :].T.astype(np.float64)
        present = (counts > 0) & (ids > 0)
        presf = present.astype(np.float64)
        safe = np.where(counts > 0, counts, 1.0)
        centers = sums / safe[:, None]
        per_inst = hsum[b].astype(np.float64) / safe
        n_inst = presf.sum()
        lv = float((per_inst * presf).sum() / max(n_inst, 1.0))
        cdiff = centers[:, None, :] - centers[None, :, :]
        csq = (cdiff * cdiff).sum(-1)
        pm = present[:, None] & present[None, :] & (ids[:, None] < ids[None, :])
        cdist = np.sqrt(np.where(pm, csq, 1.0))
        ph = np.square(np.maximum(2.0 * DELTA_DST - cdist, 0.0)) * pm
        n_pairs = pm.sum()
        ld = float(ph.sum() / max(n_pairs, 1.0))
        cn = np.sqrt(np.where(present, (centers * centers).sum(-1), 1.0))
        lr = float((cn * presf).sum() / max(n_inst, 1.0))
        valid = 1.0 if n_inst > 0 else 0.0
        lv_l.append(lv * valid)
        ld_l.append(ld * valid)
        lr_l.append(lr * valid)
        valid_l.append(valid)
    vb = max(sum(valid_l), 1.0)
    loss_var = sum(lv_l) / vb
    loss_dst = sum(ld_l) / vb
    loss_reg = sum(lr_l) / vb
    total = A_W * loss_var + B_W * loss_dst + R_W * loss_reg
    return (
        np.float32(total),
        np.float32(loss_var),
        np.float32(loss_dst),
        np.float32(loss_reg),
    )


def kernel(embedding, ins_label):
    from concourse.bass_utils import run_bass_kernel_spmd

    key = "mod"
    if key not in _cache:
        _cache[key] = build_module()
    nc = _cache[key]

    consts = _consts(512)
    emb_r = np.asarray(embedding, dtype=np.float32).reshape(B, E, NIMG)
    lab_r = np.asarray(ins_label).reshape(B, NIMG).astype(np.float32)

    in_maps = []
    for c in range(NCORES):
        b, h = c // 2, c % 2
        sl = slice(h * NPIX, (h + 1) * NPIX)
        m = dict(consts)
        m.update(_host_layouts(
            np.ascontiguousarray(emb_r[b, :, sl]),
            np.ascontiguousarray(lab_r[b, sl]), 512))
        in_maps.append(m)

    trace = bool(os.environ.get("KERNEL_TRACE"))
    res = run_bass_kernel_spmd(nc, in_maps, core_ids=list(range(NCORES)),
                               trace=trace)
    global LAST_RES
    LAST_RES = res

    stats = np.stack([res.results[2 * b]["stats"] for b in range(B)])
    hsum = np.zeros((B, K), dtype=np.float64)
    for c in range(NCORES):
        hsum[c // 2] += res.results[c]["hpart"].astype(np.float64).reshape(K)
    hsum /= SC
    return _host_finalize(stats, hsum)


if __name__ == "__main__":
    build_module()
    print("build ok")


# revision 28
# speedup vs baseline: 1.9592x; 1.2860x over previous
"""DiscriminativeLoss kernel for 8 trn2 NeuronCores (Bass/Tile), v2.

Sharding: core c handles image b = c//2, pixel half h = c%2 (N = 524288
pixels per core).  Per core:
  pass 1: per-class counts + segment sums over the pixel shard via 4096
          col-tiled fp8 matmuls (128-pixel contraction each, 4 concurrent
          PE column groups), one-hot rhs built on DVE.
  pairwise AllReduce (cores 2b, 2b+1) of the [17, 10] stats block.
  pass 2: per-pixel hinge-distance sums per class with 12 pixel lanes per
          class on 120 PSUM partitions; 3 accumulating matmuls per
          512-column chunk build 64*(s - 2*e.c_k) - 16384*(lab-k)^2 style
          values; DVE ReLU+accum and batched ACT Sqrt+accum produce the
          two per-lane moments; wrong-class lanes contribute exactly 0.
Host: builds fp8/bf16 staged layouts, sums per-core hinge partials, and
does the final ~500-flop scalar assembly.
"""

import os
import sys

import numpy as np

sys.path.insert(0, "/opt/trn_rl_repo")
os.environ.setdefault("MYCRO_LOCAL_CACHE", "1")

import ml_dtypes  # noqa: E402

BF16 = ml_dtypes.bfloat16
FP8 = ml_dtypes.float8_e4m3

# problem constants (hardcoded per harness contract)
B, E, H, W = 4, 16, 1024, 1024
NIMG = H * W
NCORES = 8
NPIX = NIMG // 2             # pixels per core
K = 10
DELTA_VAR = 0.5
DELTA_DST = 1.5
A_W, B_W, R_W = 1.0, 1.0, 0.001

J2 = 12                      # pass-2 pixel lanes per class
KJ2 = K * J2                 # 120 psum partitions, lane q = 12k + j
F2 = 512                     # pass-2 chunk columns
CHPIX = J2 * F2              # pixels per pass-2 chunk
SC = 64.0                    # center scale for fp8 stationary
BETA = 16384.0               # mask scale (= 256 * SC)
EP1 = E + 2                  # pass-1 P1 columns: 16 emb + ones + lab

_cache = {}


def _consts(f1):
    """Host-side constant input arrays shared by all cores."""
    # kpatk: [128, f1*K] fp8: kpatk[p, f*K + k] = k
    kpatk = np.tile(np.arange(K, dtype=np.float32), (128, f1)).astype(FP8)
    # W2: [37, KJ2] bf16 aux stationary: s/lab/lab^2 rows per j, then the
    # bf16-exact -BETA*k^2 mask-bias row (stream row = ones); the small
    # device-computed SC*(|c|^2 - dvar^2) bias rides a 38th device row.
    w2 = np.zeros((3 * J2 + 1, KJ2), dtype=np.float32)
    for j in range(J2):
        for k in range(K):
            q = J2 * k + j
            w2[j, q] = SC
            w2[J2 + j, q] = 2.0 * BETA * k
            w2[2 * J2 + j, q] = -BETA
    for q in range(KJ2):
        w2[3 * J2, q] = -BETA * (q // J2) ** 2
    # gt64: [K, KJ2] f32: 64 at (k, 12k+j)
    gt64 = np.zeros((K, KJ2), dtype=np.float32)
    # jcol: [KJ2, K] f32: 1 at (12k+j, k)
    jcol = np.zeros((KJ2, K), dtype=np.float32)
    for k in range(K):
        for j in range(J2):
            q = J2 * k + j
            gt64[k, q] = SC
            jcol[q, k] = 1.0
    id17 = np.eye(EP1 - 1, dtype=np.float32)
    id10 = np.eye(K, dtype=np.float32)
    id120 = np.eye(KJ2, dtype=np.float32)
    return {
        "kpatk": kpatk,
        "w2": w2.astype(BF16),
        "gt64": gt64,
        "jcol": jcol,
        "id17": id17,
        "id10": id10,
        "id120": id120,
    }


def build_module(npix=NPIX, f1=512, ncores=NCORES, pair_groups=None):
    """Build the SPMD Bass module (same program on all cores)."""
    import concourse.bass as bass  # noqa: F401
    import concourse.mybir as mybir
    import concourse.tile as tile
    from concourse import bacc

    f32 = mybir.dt.float32
    bf16 = mybir.dt.bfloat16
    f8 = mybir.dt.float8e4
    Alu = mybir.AluOpType
    Act = mybir.ActivationFunctionType

    t1 = npix // 128             # pass-1 pixel tiles
    nch1 = t1 // f1              # pass-1 chunks
    assert t1 * 128 == npix and nch1 * f1 == t1
    nch2 = -(-npix // CHPIX)     # pass-2 chunks (padded)
    n2c = nch2 * F2              # pass-2 columns per lane row
    npp = float(n2c)             # pixels per (k, j) lane
    if pair_groups is None:
        pair_groups = [[2 * i, 2 * i + 1] for i in range(ncores // 2)]

    UB = 4                       # pass-2 chunks per ACT sqrt batch
    DB = 4                       # pass-2 chunks per input DMA

    nc = bacc.Bacc(
        "TRN2",
        target_bir_lowering=False,
        debug=False,
        num_devices=ncores,
    )

    # I/O
    p1_d = nc.dram_tensor("p1", [128, t1 * EP1], f8, kind="ExternalInput").ap()
    e2a_d = nc.dram_tensor("e2a", [96, n2c], f8, kind="ExternalInput").ap()
    e2b_d = nc.dram_tensor("e2b", [96, n2c], f8, kind="ExternalInput").ap()
    aux_d = nc.dram_tensor("aux", [3 * J2 + 2, n2c], bf16,
                           kind="ExternalInput").ap()
    kpatk_d = nc.dram_tensor("kpatk", [128, f1 * K], f8, kind="ExternalInput").ap()
    w2_d = nc.dram_tensor("w2", [3 * J2 + 1, KJ2], bf16,
                          kind="ExternalInput").ap()
    gt64_d = nc.dram_tensor("gt64", [K, KJ2], f32, kind="ExternalInput").ap()
    jcol_d = nc.dram_tensor("jcol", [KJ2, K], f32, kind="ExternalInput").ap()
    id17_d = nc.dram_tensor("id17", [EP1 - 1, EP1 - 1], f32,
                            kind="ExternalInput").ap()
    id10_d = nc.dram_tensor("id10", [K, K], f32, kind="ExternalInput").ap()
    id120_d = nc.dram_tensor("id120", [KJ2, KJ2], f32,
                             kind="ExternalInput").ap()

    stats_ext = nc.dram_tensor("stats", [EP1 - 1, K], f32,
                               kind="ExternalOutput").ap()
    hpart_d = nc.dram_tensor("hpart", [1, K], f32, kind="ExternalOutput").ap()

    with tile.TileContext(nc) as tc:
        with (
            tc.tile_pool(name="consts", bufs=1) as cp,
            tc.tile_pool(name="p1", bufs=3) as p1p,
            tc.tile_pool(name="oh", bufs=3) as ohp,
            tc.tile_pool(name="p2", bufs=2) as p2p,
            tc.tile_pool(name="ub", bufs=2) as ubp,
            tc.tile_pool(name="ps2", bufs=4, space="PSUM") as psp,
            tc.tile_pool(name="ps1", bufs=1, space="PSUM") as ps1,
            tc.tile_pool(name="dram", bufs=1, space="DRAM") as dp,
        ):
            # ---- persistent constants ----
            kpatk_t = cp.tile([128, f1 * K], f8)
            nc.sync.dma_start(kpatk_t[:], kpatk_d[:])
            w2_t = cp.tile([3 * J2 + 2, KJ2], bf16)
            nc.sync.dma_start(w2_t[0:3 * J2 + 1, :], w2_d[:])
            gt64_t = cp.tile([K, KJ2], f32)
            nc.sync.dma_start(gt64_t[:], gt64_d[:])
            id120_t = cp.tile([KJ2, KJ2], f32)
            nc.sync.dma_start(id120_t[:], id120_d[:])
            jcol_t = cp.tile([KJ2, K], f32)
            nc.sync.dma_start(jcol_t[:], jcol_d[:])
            id17_t = cp.tile([EP1 - 1, EP1 - 1], f32)
            nc.sync.dma_start(id17_t[:], id17_d[:])
            id10_t = cp.tile([K, K], f32)
            nc.sync.dma_start(id10_t[:], id10_d[:])

            # ---- pass 1: counts+sums via fp8 DoubleRow one-hot matmuls ----
            psum1 = ps1.tile([EP1 - 1, K], f32)
            for c in range(nch1):
                p1c = p1p.tile([128, f1 * EP1], f8, tag="p1c")
                nc.sync.dma_start(
                    p1c[:], p1_d[:, c * f1 * EP1:(c + 1) * f1 * EP1])
                p1v = p1c[:].rearrange("p (f e) -> p f e", e=EP1)
                p1v2 = p1c[:].rearrange("p (i t e) -> p t i e", i=2, e=EP1)
                oh = ohp.tile([128, f1 * K], f8, tag="oh")
                ohv = oh[:].rearrange("p (f k) -> p f k", k=K)
                ohv2 = oh[:].rearrange("p (i t k) -> p t i k", i=2, k=K)
                nc.vector.tensor_tensor(
                    out=ohv,
                    in0=p1v[:, :, EP1 - 1:EP1].to_broadcast([128, f1, K]),
                    in1=kpatk_t[:].rearrange("p (f k) -> p f k", k=K),
                    op=Alu.is_equal,
                )
                for t in range(f1 // 2):
                    nc.tensor.matmul(
                        psum1[:],
                        lhsT=p1v2[:, t, :, 0:EP1 - 1],
                        rhs=ohv2[:, t, :, :],
                        start=(c == 0 and t == 0),
                        stop=(c == nch1 - 1 and t == f1 // 2 - 1),
                        perf_mode=mybir.MatmulPerfMode.DoubleRow,
                    )

            # ---- stats -> DRAM -> pairwise AllReduce ----
            cc_in = dp.tile([EP1 - 1, K], f32)
            cc_out = dp.tile([EP1 - 1, K], f32)
            sb1 = cp.tile([EP1 - 1, K], f32)
            nc.scalar.copy(sb1[:], psum1[:])
            nc.sync.dma_start(cc_in[:], sb1[:])
            nc.gpsimd.collective_compute(
                "AllReduce",
                mybir.AluOpType.add,
                replica_groups=pair_groups,
                ins=[cc_in[:].opt()],
                outs=[cc_out[:].opt()],
            )
            myst = cp.tile([EP1 - 1, K], f32)
            nc.sync.dma_start(myst[:], cc_out[:])
            nc.sync.dma_start(stats_ext[:], myst[:])

            # ---- centers & pass-2 stationaries ----
            ps_a = psp.tile([KJ2, F2], f32, tag="ps2", name="ps_a")
            stT_ps = ps_a[0:K, 0:EP1 - 1]
            nc.tensor.transpose(stT_ps, myst[:], id17_t[:])
            stT = cp.tile([K, EP1 - 1], f32)
            nc.scalar.copy(stT[:], stT_ps)
            cnt_safe = cp.tile([K, 1], f32)
            nc.vector.tensor_scalar(
                out=cnt_safe[:], in0=stT[:, E:E + 1], scalar1=1.0,
                scalar2=None, op0=Alu.max)
            rec = cp.tile([K, 1], f32)
            nc.vector.reciprocal(rec[:], cnt_safe[:])
            cmat = cp.tile([K, E], f32)
            nc.vector.tensor_scalar(
                out=cmat[:], in0=stT[:, 0:E], scalar1=rec[:, 0:1],
                scalar2=None, op0=Alu.mult)
            csq = cp.tile([K, E], f32)
            nc.vector.tensor_tensor(csq[:], cmat[:], cmat[:], op=Alu.mult)
            c2 = cp.tile([K, 1], f32)
            nc.vector.tensor_reduce(c2[:], csq[:], mybir.AxisListType.X, Alu.add)
            ps_b = psp.tile([KJ2, F2], f32, tag="ps2", name="ps_b")
            qb_ps = ps_b[:, 0:1]
            nc.tensor.matmul(qb_ps, lhsT=gt64_t[:], rhs=c2[:],
                             start=True, stop=True)
            qsb = cp.tile([KJ2, 1], f32)
            nc.scalar.copy(qsb[:], qb_ps)
            ps_q = psp.tile([KJ2, F2], f32, tag="ps2", name="ps_q")
            qT_ps = ps_q[0:1, 0:KJ2]
            nc.tensor.transpose(qT_ps, qsb[:], id120_t[:])
            qT = cp.tile([1, KJ2], bf16)
            nc.vector.tensor_scalar(
                out=qT[:], in0=qT_ps,
                scalar1=-SC * DELTA_VAR * DELTA_VAR, scalar2=None,
                op0=Alu.add)
            nc.sync.dma_start(w2_t[3 * J2 + 1:3 * J2 + 2, :], qT[:])

            cmm = cp.tile([K, E], f32)
            nc.vector.tensor_scalar(
                out=cmm[:], in0=cmat[:], scalar1=-2.0 * SC, scalar2=None,
                op0=Alu.mult)
            ps_c = psp.tile([KJ2, F2], f32, tag="ps2", name="ps_c")
            cT_ps = ps_c[0:E, 0:K]
            nc.tensor.transpose(cT_ps, cmm[:], id10_t[:])
            cT8 = cp.tile([E, K], f8)
            nc.scalar.copy(cT8[:], cT_ps)
            w1a = cp.tile([96, KJ2], f8)
            w1b = cp.tile([96, KJ2], f8)
            nc.vector.memset(w1a[:], 0.0)
            nc.vector.memset(w1b[:], 0.0)
            w1a_v = w1a[:].rearrange("p (k j) -> p j k", j=J2)
            w1b_v = w1b[:].rearrange("p (k j) -> p j k", j=J2)
            for j in range(6):
                nc.sync.dma_start(w1a_v[16 * j:16 * (j + 1), j, :], cT8[:])
                nc.sync.dma_start(w1b_v[16 * j:16 * (j + 1), 6 + j, :], cT8[:])

            # ---- pass 2 ----
            dv2 = cp.tile([KJ2, 1], f32)
            nc.vector.memset(dv2[:], DELTA_VAR * DELTA_VAR)
            ucol = cp.tile([KJ2, nch2], f32)
            ycol = cp.tile([KJ2, -(-nch2 // UB)], f32)
            ubuf = None
            for c in range(nch2):
                if c % DB == 0:
                    nd = min(DB, nch2 - c) * F2
                    e2a_t = p2p.tile([96, DB * F2], f8, tag="e2a")
                    nc.sync.dma_start(
                        e2a_t[:, 0:nd], e2a_d[:, c * F2:c * F2 + nd])
                    e2b_t = p2p.tile([96, DB * F2], f8, tag="e2b")
                    nc.sync.dma_start(
                        e2b_t[:, 0:nd], e2b_d[:, c * F2:c * F2 + nd])
                    aux_t = p2p.tile([3 * J2 + 2, DB * F2], bf16, tag="aux")
                    nc.sync.dma_start(
                        aux_t[:, 0:nd], aux_d[:, c * F2:c * F2 + nd])
                off = (c % DB) * F2
                ps2 = psp.tile([KJ2, F2], f32, tag="ps2")
                nc.tensor.matmul(ps2[:], lhsT=w1a[:], rhs=e2a_t[:, off:off + F2],
                                 start=True, stop=False)
                nc.tensor.matmul(ps2[:], lhsT=w1b[:], rhs=e2b_t[:, off:off + F2],
                                 start=False, stop=False)
                nc.tensor.matmul(ps2[:], lhsT=w2_t[:], rhs=aux_t[:, off:off + F2],
                                 start=False, stop=True)
                if c % UB == 0:
                    ubuf = ubp.tile([KJ2, UB * F2], bf16, tag="ubuf")
                uo = (c % UB) * F2
                nc.vector.tensor_scalar(
                    out=ubuf[:, uo:uo + F2],
                    in0=ps2[:],
                    scalar1=0.0,
                    scalar2=None,
                    op0=Alu.max,
                    op1=Alu.add,
                    accum_out=ucol[:, c:c + 1],
                )
                if c % UB == UB - 1 or c == nch2 - 1:
                    nu = (c % UB + 1) * F2
                    ytr = ubp.tile([KJ2, UB * F2], bf16, tag="ytr")
                    nc.scalar.activation(
                        ytr[:, 0:nu], ubuf[:, 0:nu], Act.Sqrt,
                        bias=dv2[:, 0:1], scale=1.0 / SC,
                        accum_out=ycol[:, c // UB:c // UB + 1])

            # ---- H assembly ----
            u1 = cp.tile([KJ2, 1], f32)
            y1 = cp.tile([KJ2, 1], f32)
            nc.vector.tensor_reduce(u1[:], ucol[:], mybir.AxisListType.X, Alu.add)
            nc.vector.tensor_reduce(y1[:], ycol[:], mybir.AxisListType.X, Alu.add)
            hp = cp.tile([KJ2, 1], f32)
            nc.vector.scalar_tensor_tensor(
                out=hp[:], in0=y1[:], scalar=-2.0 * DELTA_VAR * SC, in1=u1[:],
                op0=Alu.mult, op1=Alu.add)
            hp2 = cp.tile([KJ2, 1], f32)
            nc.vector.tensor_scalar(
                out=hp2[:], in0=hp[:],
                scalar1=2.0 * DELTA_VAR * DELTA_VAR * SC * npp,
                scalar2=None, op0=Alu.add)
            ps_h = psp.tile([KJ2, F2], f32, tag="ps2", name="ps_h")
            h_ps = ps_h[0:1, 0:K]
            nc.tensor.matmul(h_ps, lhsT=hp2[:], rhs=jcol_t[:],
                             start=True, stop=True)
            h_sb = cp.tile([1, K], f32)
            nc.scalar.copy(h_sb[:], h_ps)
            nc.sync.dma_start(hpart_d[:], h_sb[:])

    nc.compile()
    return nc


def _host_layouts(e_half, lab_half, f1):
    """Build per-core staged arrays: P1 fp8, e2a/e2b fp8, aux bf16."""
    npix = e_half.shape[1]
    t1 = npix // 128
    nch2 = -(-npix // CHPIX)
    n2pad = nch2 * CHPIX - npix

    p1 = np.empty((128, t1, EP1), dtype=FP8)
    p1[:, :, 0:E] = e_half.reshape(E, t1, 128).transpose(2, 1, 0)
    p1[:, :, E] = 1.0
    p1[:, :, E + 1] = lab_half.reshape(t1, 128).T

    e_pad = np.concatenate(
        [e_half, np.zeros((E, n2pad), np.float32)], axis=1)
    y = e_pad.reshape(E, nch2, J2, F2)
    e2a = np.ascontiguousarray(
        y[:, :, 0:6].transpose(2, 0, 1, 3).reshape(96, nch2 * F2)).astype(FP8)
    e2b = np.ascontiguousarray(
        y[:, :, 6:12].transpose(2, 0, 1, 3).reshape(96, nch2 * F2)).astype(FP8)

    s = np.square(e_half).sum(axis=0)
    aux = np.empty((3 * J2 + 2, nch2, F2), dtype=np.float32)
    for i, vec in enumerate((s, lab_half, lab_half * lab_half)):
        v = np.concatenate([vec, np.zeros(n2pad, np.float32)])
        aux[i * J2:(i + 1) * J2] = v.reshape(nch2, J2, F2).transpose(1, 0, 2)
    aux[3 * J2:] = 1.0
    return {
        "p1": p1.reshape(128, t1 * EP1),
        "e2a": e2a,
        "e2b": e2b,
        "aux": aux.reshape(3 * J2 + 2, nch2 * F2).astype(BF16),
    }


def _host_finalize(stats, hsum):
    """stats: [B, 17, 10] (rows 0..15 sums[e,k], row 16 counts);
    hsum: [B, 10] hinge sums (already /SC)."""
    lv_l, ld_l, lr_l, valid_l = [], [], [], []
    ids = np.arange(K)
    for b in range(B):
        counts = stats[b, E, :].astype(np.float64)
        sums = stats[b, 0:E, :].T.astype(np.float64)
        present = (counts > 0) & (ids > 0)
        presf = present.astype(np.float64)
        safe = np.where(counts > 0, counts, 1.0)
        centers = sums / safe[:, None]
        per_inst = hsum[b].astype(np.float64) / safe
        n_inst = presf.sum()
        lv = float((per_inst * presf).sum() / max(n_inst, 1.0))
        cdiff = centers[:, None, :] - centers[None, :, :]
        csq = (cdiff * cdiff).sum(-1)
        pm = present[:, None] & present[None, :] & (ids[:, None] < ids[None, :])
        cdist = np.sqrt(np.where(pm, csq, 1.0))
        ph = np.square(np.maximum(2.0 * DELTA_DST - cdist, 0.0)) * pm
        n_pairs = pm.sum()
        ld = float(ph.sum() / max(n_pairs, 1.0))
        cn = np.sqrt(np.where(present, (centers * centers).sum(-1), 1.0))
        lr = float((cn * presf).sum() / max(n_inst, 1.0))
        valid = 1.0 if n_inst > 0 else 0.0
        lv_l.append(lv * valid)
        ld_l.append(ld * valid)
        lr_l.append(lr * valid)
        valid_l.append(valid)
    vb = max(sum(valid_l), 1.0)
    loss_var = sum(lv_l) / vb
    loss_dst = sum(ld_l) / vb
    loss_reg = sum(lr_l) / vb
    total = A_W * loss_var + B_W * loss_dst + R_W * loss_reg
    return (
        np.float32(total),
        np.float32(loss_var),
        np.float32(loss_dst),
        np.float32(loss_reg),
    )


def kernel(embedding, ins_label):
    from concourse.bass_utils import run_bass_kernel_spmd

    key = "mod"
    if key not in _cache:
        _cache[key] = build_module()
    nc = _cache[key]

    consts = _consts(512)
    emb_r = np.asarray(embedding, dtype=np.float32).reshape(B, E, NIMG)
    lab_r = np.asarray(ins_label).reshape(B, NIMG).astype(np.float32)

    in_maps = []
    for c in range(NCORES):
        b, h = c // 2, c % 2
        sl = slice(h * NPIX, (h + 1) * NPIX)
        m = dict(consts)
        m.update(_host_layouts(
            np.ascontiguousarray(emb_r[b, :, sl]),
            np.ascontiguousarray(lab_r[b, sl]), 512))
        in_maps.append(m)

    trace = bool(os.environ.get("KERNEL_TRACE"))
    res = run_bass_kernel_spmd(nc, in_maps, core_ids=list(range(NCORES)),
                               trace=trace)
    global LAST_RES
    LAST_RES = res

    stats = np.stack([res.results[2 * b]["stats"] for b in range(B)])
    hsum = np.zeros((B, K), dtype=np.float64)
    for c in range(NCORES):
        hsum[c // 2] += res.results[c]["hpart"].astype(np.float64).reshape(K)
    hsum /= SC
    return _host_finalize(stats, hsum)


if __name__ == "__main__":
    build_module()
    print("build ok")


# revision 51
# speedup vs baseline: 2.2330x; 1.1398x over previous
"""DiscriminativeLoss kernel for 8 trn2 NeuronCores (Bass/Tile), v2.

Sharding: core c handles image b = c//2, pixel half h = c%2 (N = 524288
pixels per core).  Per core:
  pass 1: per-class counts + segment sums over the pixel shard via 4096
          col-tiled fp8 matmuls (128-pixel contraction each, 4 concurrent
          PE column groups), one-hot rhs built on DVE.
  pairwise AllReduce (cores 2b, 2b+1) of the [17, 10] stats block.
  pass 2: per-pixel hinge-distance sums per class with 12 pixel lanes per
          class on 120 PSUM partitions; 3 accumulating matmuls per
          512-column chunk build 64*(s - 2*e.c_k) - 16384*(lab-k)^2 style
          values; DVE ReLU+accum and batched ACT Sqrt+accum produce the
          two per-lane moments; wrong-class lanes contribute exactly 0.
Host: builds fp8/bf16 staged layouts, sums per-core hinge partials, and
does the final ~500-flop scalar assembly.
"""

import os
import sys

import numpy as np

sys.path.insert(0, "/opt/trn_rl_repo")
os.environ.setdefault("MYCRO_LOCAL_CACHE", "1")

import ml_dtypes  # noqa: E402

BF16 = ml_dtypes.bfloat16
FP8 = ml_dtypes.float8_e4m3

# problem constants (hardcoded per harness contract)
B, E, H, W = 4, 16, 1024, 1024
NIMG = H * W
NCORES = 8
NPIX = NIMG // 2             # pixels per core
K = 10
DELTA_VAR = 0.5
DELTA_DST = 1.5
A_W, B_W, R_W = 1.0, 1.0, 0.001

J2 = 12                      # pass-2 pixel lanes per class
KJ2 = K * J2                 # 120 psum partitions, lane q = 12k + j
F2 = 512                     # pass-2 chunk columns
CHPIX = J2 * F2              # pixels per pass-2 chunk
SC = 64.0                    # center scale for fp8 stationary
BETA = 16384.0               # mask scale (= 256 * SC)
EP1 = E + 2                  # pass-1 P1 columns: 16 emb + ones + lab

_cache = {}


def _consts(f1):
    """Host-side constant input arrays shared by all cores."""
    # kpatk: [128, f1*K] fp8: kpatk[p, f*K + k] = k
    kpatk = np.tile(np.arange(K, dtype=np.float32), (128, f1)).astype(FP8)
    # W2: [37, KJ2] bf16 aux stationary: s/lab/lab^2 rows per j, then the
    # bf16-exact -BETA*k^2 mask-bias row (stream row = ones); the small
    # device-computed SC*(|c|^2 - dvar^2) bias rides a 38th device row.
    w2 = np.zeros((3 * J2 + 1, KJ2), dtype=np.float32)
    for j in range(J2):
        for k in range(K):
            q = J2 * k + j
            w2[j, q] = SC
            w2[J2 + j, q] = 2.0 * BETA * k
            w2[2 * J2 + j, q] = -BETA
    for q in range(KJ2):
        w2[3 * J2, q] = -BETA * (q // J2) ** 2
    # gt64: [K, KJ2] f32: 64 at (k, 12k+j)
    gt64 = np.zeros((K, KJ2), dtype=np.float32)
    # jcol: [KJ2, K] f32: 1 at (12k+j, k)
    jcol = np.zeros((KJ2, K), dtype=np.float32)
    for k in range(K):
        for j in range(J2):
            q = J2 * k + j
            gt64[k, q] = SC
            jcol[q, k] = 1.0
    id17 = np.eye(EP1 - 1, dtype=np.float32)
    id10 = np.eye(K, dtype=np.float32)
    id120 = np.eye(KJ2, dtype=np.float32)
    return {
        "kpatk": kpatk,
        "w2": w2.astype(BF16),
        "gt64": gt64,
        "jcol": jcol,
        "id17": id17,
        "id10": id10,
        "id120": id120,
    }


def _nch2(npix):
    n = -(-npix // CHPIX)
    return n + (n % 2)


def build_module(npix=NPIX, f1=512, ncores=NCORES, pair_groups=None):
    """Build the SPMD Bass module (same program on all cores)."""
    import concourse.bass as bass  # noqa: F401
    import concourse.mybir as mybir
    import concourse.tile as tile
    from concourse import bacc

    f32 = mybir.dt.float32
    bf16 = mybir.dt.bfloat16
    f8 = mybir.dt.float8e4
    Alu = mybir.AluOpType
    Act = mybir.ActivationFunctionType

    t1 = npix // 128             # pass-1 pixel tiles
    nch1 = t1 // f1              # pass-1 chunks
    assert t1 * 128 == npix and nch1 * f1 == t1
    nch2 = _nch2(npix)           # pass-2 chunks (padded, even)
    n2c = nch2 * F2              # pass-2 columns per lane row
    npp = float(n2c)             # pixels per (k, j) lane
    if pair_groups is None:
        pair_groups = [[2 * i, 2 * i + 1] for i in range(ncores // 2)]

    UB = 4                       # pass-2 chunks per ACT sqrt batch
    DB = 4                       # pass-2 chunks per input DMA
    NJUNK = 0                    # PE warm-keeper matmuls through the gap

    nc = bacc.Bacc(
        "TRN2",
        target_bir_lowering=False,
        debug=False,
        num_devices=ncores,
    )

    # I/O
    p1_d = nc.dram_tensor("p1", [128, t1 * EP1], f8, kind="ExternalInput").ap()
    e2ab_d = nc.dram_tensor("e2ab", [96, 2 * n2c], f8, kind="ExternalInput").ap()
    aux_d = nc.dram_tensor("aux", [3 * J2 + 2, n2c], bf16,
                           kind="ExternalInput").ap()
    kpatk_d = nc.dram_tensor("kpatk", [128, f1 * K], f8, kind="ExternalInput").ap()
    w2_d = nc.dram_tensor("w2", [3 * J2 + 1, KJ2], bf16,
                          kind="ExternalInput").ap()
    gt64_d = nc.dram_tensor("gt64", [K, KJ2], f32, kind="ExternalInput").ap()
    jcol_d = nc.dram_tensor("jcol", [KJ2, K], f32, kind="ExternalInput").ap()
    id17_d = nc.dram_tensor("id17", [EP1 - 1, EP1 - 1], f32,
                            kind="ExternalInput").ap()
    id10_d = nc.dram_tensor("id10", [K, K], f32, kind="ExternalInput").ap()
    id120_d = nc.dram_tensor("id120", [KJ2, KJ2], f32,
                             kind="ExternalInput").ap()

    stats_ext = nc.dram_tensor("stats", [EP1 - 1, K], f32,
                               kind="ExternalOutput").ap()
    hpart_d = nc.dram_tensor("hpart", [1, K], f32, kind="ExternalOutput").ap()

    with tile.TileContext(nc) as tc:
        with (
            tc.tile_pool(name="consts", bufs=1) as cp,
            tc.tile_pool(name="p1", bufs=3) as p1p,
            tc.tile_pool(name="oh", bufs=3) as ohp,
            tc.tile_pool(name="p2", bufs=2) as p2p,
            tc.tile_pool(name="ub", bufs=2) as ubp,
            tc.tile_pool(name="ps2", bufs=6, space="PSUM") as psp,
            tc.tile_pool(name="ps1", bufs=1, space="PSUM") as ps1,
            tc.tile_pool(name="dram", bufs=1, space="DRAM") as dp,
        ):
            # ---- persistent constants ----
            kpatk_t = cp.tile([128, f1 * K], f8)
            nc.sync.dma_start(kpatk_t[:], kpatk_d[:])
            w2_t = cp.tile([3 * J2 + 2, KJ2], bf16)
            nc.sync.dma_start(w2_t[0:3 * J2 + 1, :], w2_d[:])
            gt64_t = cp.tile([K, KJ2], f32)
            nc.sync.dma_start(gt64_t[:], gt64_d[:])
            id120_t = cp.tile([KJ2, KJ2], f32)
            nc.sync.dma_start(id120_t[:], id120_d[:])
            jcol_t = cp.tile([KJ2, K], f32)
            nc.sync.dma_start(jcol_t[:], jcol_d[:])
            id17_t = cp.tile([EP1 - 1, EP1 - 1], f32)
            nc.sync.dma_start(id17_t[:], id17_d[:])
            id10_t = cp.tile([K, K], f32)
            nc.sync.dma_start(id10_t[:], id10_d[:])

            # ---- pass 1: counts+sums via fp8 DoubleRow one-hot matmuls ----
            psum1 = ps1.tile([EP1 - 1, K], f32)
            for c in range(nch1):
                p1c = p1p.tile([128, f1 * EP1], f8, tag="p1c")
                nc.sync.dma_start(
                    p1c[:], p1_d[:, c * f1 * EP1:(c + 1) * f1 * EP1])
                p1v = p1c[:].rearrange("p (f e) -> p f e", e=EP1)
                p1v2 = p1c[:].rearrange("p (i t e) -> p t i e", i=2, e=EP1)
                oh = ohp.tile([128, f1 * K], f8, tag="oh")
                ohv = oh[:].rearrange("p (f k) -> p f k", k=K)
                ohv2 = oh[:].rearrange("p (i t k) -> p t i k", i=2, k=K)
                hf = f1 // 2
                for s in range(2):
                    nc.vector.tensor_tensor(
                        out=ohv[:, s * hf:(s + 1) * hf, :],
                        in0=p1v[:, s * hf:(s + 1) * hf,
                                EP1 - 1:EP1].to_broadcast([128, hf, K]),
                        in1=kpatk_t[:].rearrange(
                            "p (f k) -> p f k", k=K)[:, s * hf:(s + 1) * hf, :],
                        op=Alu.is_equal,
                    )
                for t in range(f1 // 2):
                    nc.tensor.matmul(
                        psum1[:],
                        lhsT=p1v2[:, t, :, 0:EP1 - 1],
                        rhs=ohv2[:, t, :, :],
                        start=(c == 0 and t == 0),
                        stop=(c == nch1 - 1 and t == f1 // 2 - 1),
                        perf_mode=mybir.MatmulPerfMode.DoubleRow,
                    )

            # ---- stats -> DRAM -> pairwise AllReduce ----
            cc_in = dp.tile([EP1 - 1, K], f32)
            cc_out = dp.tile([EP1 - 1, K], f32)
            sb1 = cp.tile([EP1 - 1, K], f32)
            nc.scalar.copy(sb1[:], psum1[:])
            nc.sync.dma_start(cc_in[:], sb1[:])
            nc.gpsimd.collective_compute(
                "AllReduce",
                mybir.AluOpType.add,
                replica_groups=pair_groups,
                ins=[cc_in[:].opt()],
                outs=[cc_out[:].opt()],
            )
            myst = cp.tile([EP1 - 1, K], f32)
            nc.sync.dma_start(myst[:], cc_out[:])
            nc.sync.dma_start(stats_ext[:], myst[:])

            # ---- centers & pass-2 stationaries ----
            ps_a = psp.tile([128, F2], f32, tag="ps2", name="ps_a")
            stT_ps = ps_a[0:K, 0:EP1 - 1]
            nc.tensor.transpose(stT_ps, myst[:], id17_t[:])
            stT = cp.tile([K, EP1 - 1], f32)
            nc.scalar.copy(stT[:], stT_ps)
            cnt_safe = cp.tile([K, 1], f32)
            nc.vector.tensor_scalar(
                out=cnt_safe[:], in0=stT[:, E:E + 1], scalar1=1.0,
                scalar2=None, op0=Alu.max)
            rec = cp.tile([K, 1], f32)
            nc.vector.reciprocal(rec[:], cnt_safe[:])
            cmat = cp.tile([K, E], f32)
            nc.vector.tensor_scalar(
                out=cmat[:], in0=stT[:, 0:E], scalar1=rec[:, 0:1],
                scalar2=None, op0=Alu.mult)
            csq = cp.tile([K, E], f32)
            nc.vector.tensor_tensor(csq[:], cmat[:], cmat[:], op=Alu.mult)
            c2 = cp.tile([K, 1], f32)
            nc.vector.tensor_reduce(c2[:], csq[:], mybir.AxisListType.X, Alu.add)
            ps_b = psp.tile([128, F2], f32, tag="ps2", name="ps_b")
            qb_ps = ps_b[0:KJ2, 0:1]
            nc.tensor.matmul(qb_ps, lhsT=gt64_t[:], rhs=c2[:],
                             start=True, stop=True)
            qsb = cp.tile([KJ2, 1], f32)
            nc.scalar.copy(qsb[:], qb_ps)
            ps_q = psp.tile([128, F2], f32, tag="ps2", name="ps_q")
            qT_ps = ps_q[0:1, 0:KJ2]
            nc.tensor.transpose(qT_ps, qsb[:], id120_t[:])
            qT = cp.tile([1, KJ2], bf16)
            nc.vector.tensor_scalar(
                out=qT[:], in0=qT_ps,
                scalar1=-SC * DELTA_VAR * DELTA_VAR, scalar2=None,
                op0=Alu.add)
            nc.sync.dma_start(w2_t[3 * J2 + 1:3 * J2 + 2, :], qT[:])

            cmm = cp.tile([K, E], f32)
            nc.vector.tensor_scalar(
                out=cmm[:], in0=cmat[:], scalar1=-2.0 * SC, scalar2=None,
                op0=Alu.mult)
            ps_c = psp.tile([128, F2], f32, tag="ps2", name="ps_c")
            cT_ps = ps_c[0:E, 0:K]
            nc.tensor.transpose(cT_ps, cmm[:], id10_t[:])
            cT8 = cp.tile([E, K], f8)
            nc.scalar.copy(cT8[:], cT_ps)
            # w1ab: [96, (i, 128)] fp8 DoubleRow stationary; lane q = 12k+j,
            # pair i=0 covers j=0..5, i=1 covers j=6..11; cols 120..127 zero.
            w1ab = cp.tile([96, 2 * 128], f8)
            nc.vector.memset(w1ab[:], 0.0)
            w1ab_v = w1ab[:].rearrange("p (i q) -> p i q", i=2)
            issuers = [nc.sync, nc.scalar, nc.gpsimd]
            for j in range(6):
                dst_a = w1ab_v[16 * j:16 * (j + 1), 0,
                               0:KJ2].rearrange("e (k j2) -> e k j2", j2=J2)
                issuers[j % 3].dma_start(dst_a[:, :, j], cT8[:])
                dst_b = w1ab_v[16 * j:16 * (j + 1), 1,
                               0:KJ2].rearrange("e (k j2) -> e k j2", j2=J2)
                issuers[(j + 1) % 3].dma_start(dst_b[:, :, 6 + j], cT8[:])

            # ---- pass 2: per-chunk matmuls into 3 double-bank psum tiles;
            # DVE max+accum reads 2 banks per op, ACT Sqrt per 2 DVE ops ----
            dv2 = cp.tile([KJ2, 1], f32)
            nc.vector.memset(dv2[:], DELTA_VAR * DELTA_VAR)
            ucol = cp.tile([KJ2, nch2], f32)
            ycol = cp.tile([KJ2, -(-nch2 // UB)], f32)
            e2ab_r = e2ab_d[:].rearrange("p (i n) -> p i n", i=2)
            w1ab_v2 = w1ab[:].rearrange("p (i q) -> p i q", i=2)
            ubuf = None
            for c in range(nch2):
                if c % DB == 0:
                    nd = min(DB, nch2 - c) * F2
                    e2ab_t = p2p.tile([96, 2 * DB * F2], f8, tag="e2ab")
                    e2ab_tv = e2ab_t[:].rearrange("p (i n) -> p i n", i=2)
                    nc.sync.dma_start(
                        e2ab_tv[:, :, 0:nd],
                        e2ab_r[:, :, c * F2:c * F2 + nd])
                    aux_t = p2p.tile([3 * J2 + 2, DB * F2], bf16, tag="aux")
                    nc.sync.dma_start(
                        aux_t[:, 0:nd], aux_d[:, c * F2:c * F2 + nd])
                off = (c % DB) * F2
                ps2 = psp.tile([128, F2], f32, tag="ps2")
                nc.tensor.matmul(
                    ps2[:],
                    lhsT=w1ab_v2,
                    rhs=e2ab_tv[:, :, off:off + F2],
                    start=True, stop=False,
                    perf_mode=mybir.MatmulPerfMode.DoubleRow)
                nc.tensor.matmul(
                    ps2[0:KJ2, :], lhsT=w2_t[:],
                    rhs=aux_t[:, off:off + F2],
                    start=False, stop=True)
                if c % UB == 0:
                    ubuf = ubp.tile([KJ2, UB * F2], bf16, tag="ubuf")
                uo = (c % UB) * F2
                nc.vector.tensor_scalar(
                    out=ubuf[:, uo:uo + F2],
                    in0=ps2[0:KJ2, :],
                    scalar1=0.0,
                    scalar2=None,
                    op0=Alu.max,
                    op1=Alu.add,
                    accum_out=ucol[:, c:c + 1],
                )
                if c % UB == UB - 1 or c == nch2 - 1:
                    nu = (c % UB + 1) * F2
                    ytr = ubp.tile([KJ2, UB * F2], bf16, tag="ytr")
                    nc.scalar.activation(
                        ytr[:, 0:nu], ubuf[:, 0:nu], Act.Sqrt,
                        bias=dv2[:, 0:1], scale=1.0 / SC,
                        accum_out=ycol[:, c // UB:c // UB + 1])

            # ---- H assembly ----
            u1 = cp.tile([KJ2, 1], f32)
            y1 = cp.tile([KJ2, 1], f32)
            nc.vector.tensor_reduce(u1[:], ucol[:], mybir.AxisListType.X, Alu.add)
            nc.vector.tensor_reduce(y1[:], ycol[:], mybir.AxisListType.X, Alu.add)
            hp = cp.tile([KJ2, 1], f32)
            nc.vector.scalar_tensor_tensor(
                out=hp[:], in0=y1[:], scalar=-2.0 * DELTA_VAR * SC, in1=u1[:],
                op0=Alu.mult, op1=Alu.add)
            hp2 = cp.tile([KJ2, 1], f32)
            nc.vector.tensor_scalar(
                out=hp2[:], in0=hp[:],
                scalar1=2.0 * DELTA_VAR * DELTA_VAR * SC * npp,
                scalar2=None, op0=Alu.add)
            ps_h = psp.tile([128, F2], f32, tag="ps2", name="ps_h")
            h_ps = ps_h[0:1, 0:K]
            nc.tensor.matmul(h_ps, lhsT=hp2[:], rhs=jcol_t[:],
                             start=True, stop=True)
            h_sb = cp.tile([1, K], f32)
            nc.scalar.copy(h_sb[:], h_ps)
            nc.sync.dma_start(hpart_d[:], h_sb[:])

    nc.compile()
    return nc


def _host_layouts(e_half, lab_half, f1):
    """Build per-core staged arrays: P1 fp8, e2a/e2b fp8, aux bf16."""
    npix = e_half.shape[1]
    t1 = npix // 128
    nch2 = _nch2(npix)
    n2pad = nch2 * CHPIX - npix

    p1 = np.empty((128, t1, EP1), dtype=FP8)
    p1[:, :, 0:E] = e_half.reshape(E, t1, 128).transpose(2, 1, 0)
    p1[:, :, E] = 1.0
    p1[:, :, E + 1] = lab_half.reshape(t1, 128).T

    e_pad = np.concatenate(
        [e_half, np.zeros((E, n2pad), np.float32)], axis=1)
    y = e_pad.reshape(E, nch2, J2, F2)
    e2ab = np.empty((96, 2, nch2 * F2), dtype=FP8)
    e2ab[:, 0, :] = y[:, :, 0:6].transpose(2, 0, 1, 3).reshape(96, nch2 * F2)
    e2ab[:, 1, :] = y[:, :, 6:12].transpose(2, 0, 1, 3).reshape(96, nch2 * F2)

    s = np.square(e_half).sum(axis=0)
    aux = np.empty((3 * J2 + 2, nch2, F2), dtype=np.float32)
    for i, vec in enumerate((s, lab_half, lab_half * lab_half)):
        v = np.concatenate([vec, np.zeros(n2pad, np.float32)])
        aux[i * J2:(i + 1) * J2] = v.reshape(nch2, J2, F2).transpose(1, 0, 2)
    aux[3 * J2:] = 1.0
    return {
        "p1": p1.reshape(128, t1 * EP1),
        "e2ab": e2ab.reshape(96, 2 * nch2 * F2),
        "aux": aux.reshape(3 * J2 + 2, nch2 * F2).astype(BF16),
    }


def _host_finalize(stats, hsum):
    """stats: [B, 17, 10] (rows 0..15 sums[e,k], row 16 counts);
    hsum: [B, 10] hinge sums (already /SC)."""
    lv_l, ld_l, lr_l, valid_l = [], [], [], []
    ids = np.arange(K)
    for b in range(B):
        counts = stats[b, E, :].astype(np.float64)
        sums = stats[b, 0:E, :].T.astype(np.float64)
        present = (counts > 0) & (ids > 0)
        presf = present.astype(np.float64)
        safe = np.where(counts > 0, counts, 1.0)
        centers = sums / safe[:, None]
        per_inst = hsum[b].astype(np.float64) / safe
        n_inst = presf.sum()
        lv = float((per_inst * presf).sum() / max(n_inst, 1.0))
        cdiff = centers[:, None, :] - centers[None, :, :]
        csq = (cdiff * cdiff).sum(-1)
        pm = present[:, None] & present[None, :] & (ids[:, None] < ids[None, :])
        cdist = np.sqrt(np.where(pm, csq, 1.0))
        ph = np.square(np.maximum(2.0 * DELTA_DST - cdist, 0.0)) * pm
        n_pairs = pm.sum()
        ld = float(ph.sum() / max(n_pairs, 1.0))
        cn = np.sqrt(np.where(present, (centers * centers).sum(-1), 1.0))
        lr = float((cn * presf).sum() / max(n_inst, 1.0))
        valid = 1.0 if n_inst > 0 else 0.0
        lv_l.append(lv * valid)
        ld_l.append(ld * valid)
        lr_l.append(lr * valid)
        valid_l.append(valid)
    vb = max(sum(valid_l), 1.0)
    loss_var = sum(lv_l) / vb
    loss_dst = sum(ld_l) / vb
    loss_reg = sum(lr_l) / vb
    total = A_W * loss_var + B_W * loss_dst + R_W * loss_reg
    return (
        np.float32(total),
        np.float32(loss_var),
        np.float32(loss_dst),
        np.float32(loss_reg),
    )


def kernel(embedding, ins_label):
    from concourse.bass_utils import run_bass_kernel_spmd

    key = "mod"
    if key not in _cache:
        _cache[key] = build_module()
    nc = _cache[key]

    consts = _consts(512)
    emb_r = np.asarray(embedding, dtype=np.float32).reshape(B, E, NIMG)
    lab_r = np.asarray(ins_label).reshape(B, NIMG).astype(np.float32)

    in_maps = []
    for c in range(NCORES):
        b, h = c // 2, c % 2
        sl = slice(h * NPIX, (h + 1) * NPIX)
        m = dict(consts)
        m.update(_host_layouts(
            np.ascontiguousarray(emb_r[b, :, sl]),
            np.ascontiguousarray(lab_r[b, sl]), 512))
        in_maps.append(m)

    trace = bool(os.environ.get("KERNEL_TRACE"))
    res = run_bass_kernel_spmd(nc, in_maps, core_ids=list(range(NCORES)),
                               trace=trace)
    global LAST_RES
    LAST_RES = res

    stats = np.stack([res.results[2 * b]["stats"] for b in range(B)])
    hsum = np.zeros((B, K), dtype=np.float64)
    for c in range(NCORES):
        hsum[c // 2] += res.results[c]["hpart"].astype(np.float64).reshape(K)
    hsum /= SC
    return _host_finalize(stats, hsum)


if __name__ == "__main__":
    build_module()
    print("build ok")
